# revision 1
# baseline (speedup 1.0000x reference)
"""Trainium2 Bass kernel for BaseGINE (4-layer GINE message-passing GNN).

Self-contained: takes full inputs, shards across 8 NeuronCores internally,
returns the full output.

Sharding: nodes are partitioned contiguously across the 8 cores (12500 each,
padded to 12544 = 98*128). Edges are assigned to the core owning their dst
node and sorted into (src-chunk, dst-window) cells:
  - src-chunk: quarter of the padded node table holding the src row
    (dma_gather indices are int16, so the 100352-row table is read in 4
    chunks of 25088 rows)
  - dst-window: 128-node window of the destination, so each 128-edge tile
    feeds one PSUM-accumulated indicator matmul
Per layer, per superwindow (4 dst windows): dma_gather x[src] for the 4
chunk batches -> msg = relu(x_src + e_emb) (bf16) -> window-major per-tile
matmuls with a host-built weighted indicator (edge_weight folded in, bf16)
accumulate aggT[d, window] in one PSUM bank -> h = (1+eps)x + agg ->
MLP/BN/relu/residual on the x^T-resident fp32 slice. Layer end: PE-transpose
x^T to rows -> AllGather rebuilds the gather table for the next layer.
"""

import numpy as np

import concourse.bass as bass
import concourse.bacc as bacc
import concourse.mybir as mybir
import concourse.tile as tile
from concourse.bass_utils import run_bass_kernel_spmd
from concourse.masks import make_identity

F32 = mybir.dt.float32
BF16 = mybir.dt.bfloat16
I16 = mybir.dt.int16

NCORES = 8
D = 128
ED = 16
L = 4
P = 128
WIN = 128            # dst-window width (nodes)
SW_WINS = 4          # windows per superwindow (one PSUM bank)
NCHUNK = 4           # gather-table chunks (int16 index limit)
GTILES = 24          # max tiles per dma_gather instruction
BN_EPS = 1e-5
EMB_DT = BF16        # e_emb storage dtype
IND_DT = BF16        # indicator dtype (matches msg dtype for matmul)
MSG_DT = BF16        # msg dtype (agg matmul operands)
TAB_DT = BF16        # gather table dtype


# ---------------------------------------------------------------------------
# host-side prep
# ---------------------------------------------------------------------------

def _prep(x, edge_index, edge_attr, edge_weight, n_nodes):
    """Sort/pad edges into the (chunk, window) cell structure."""
    nlr = n_nodes // NCORES              # real nodes per core
    nl = -(-nlr // P) * P                # padded nodes per core
    ntab = nl * NCORES                   # padded table rows
    crows = ntab // NCHUNK               # table rows per chunk
    assert crows <= 32768
    nwin = nl // WIN                     # windows per core

    src, dst = edge_index[0].astype(np.int64), edge_index[1].astype(np.int64)
    core = dst // nlr
    ldst = dst - core * nlr
    win = ldst // WIN
    spos = (src // nlr) * nl + (src % nlr)   # src row in padded table
    q = spos // crows

    cell_edges = {}
    counts = np.zeros((NCORES, NCHUNK, nwin), np.int64)
    order = np.lexsort((win, q, core))
    sc, sq, sw_ = core[order], q[order], win[order]
    bounds = np.flatnonzero(
        np.r_[True, (sc[1:] != sc[:-1]) | (sq[1:] != sq[:-1]) | (sw_[1:] != sw_[:-1])])
    bounds = np.r_[bounds, len(order)]
    for b0, b1 in zip(bounds[:-1], bounds[1:]):
        cell_edges[(sc[b0], sq[b0], sw_[b0])] = order[b0:b1]
        counts[sc[b0], sq[b0], sw_[b0]] = b1 - b0

    # uniform tiles per cell: max over cores, at least 1
    T = np.maximum(1, -(-counts.max(axis=0) // P))      # [NCHUNK, nwin]

    n_sw = -(-nwin // SW_WINS)
    # stream order: superwindow -> chunk -> window
    stream = []
    for s in range(n_sw):
        wlo, whi = s * SW_WINS, min((s + 1) * SW_WINS, nwin)
        for qq in range(NCHUNK):
            assert T[qq, wlo:whi].sum() <= GTILES, "raise GTILES"
            for ww in range(wlo, whi):
                stream.append((qq, ww))
    ntiles = int(T.sum())
    epad = ntiles * P

    idx16 = np.zeros((NCORES, 128, epad // 16), np.int16)
    ind = np.zeros((NCORES, P, ntiles, WIN), np.float32)
    eaT = np.zeros((NCORES, ED + 1, epad), np.float32)

    tile_win = np.zeros(ntiles, np.int64)
    tile_start = np.zeros(ntiles, np.bool_)
    tile_stop = np.zeros(ntiles, np.bool_)

    t0 = 0
    for (qq, ww) in stream:
        nt = int(T[qq, ww])
        tile_win[t0:t0 + nt] = ww
        if qq == 0:
            tile_start[t0] = True
        if qq == NCHUNK - 1:
            tile_stop[t0 + nt - 1] = True
        for c in range(NCORES):
            e = cell_edges.get((c, qq, ww), np.empty(0, np.int64))
            k = len(e)
            s0 = t0 * P
            loc = np.zeros(nt * P, np.int64)
            loc[:k] = spos[e] - qq * crows
            i = np.arange(nt * P)
            idx16[c, i % 16, (s0 + i) // 16] = loc.astype(np.int16)
            tt, pp = i[:k] // P, i[:k] % P
            ind[c, pp, t0 + tt, (ldst[e] - ww * WIN)] = edge_weight[e]
            eaT[c, :ED, s0:s0 + k] = edge_attr[e].T
            eaT[c, ED, s0:s0 + k] = 1.0
        t0 += nt
    assert t0 == ntiles
    for g in range(1, 8):
        idx16[:, g * 16:(g + 1) * 16, :] = idx16[:, :16, :]

    # one gather batch per (superwindow, chunk)
    batches = []   # (q, n_tiles, tile_lo)
    t0 = 0
    for s in range(n_sw):
        wlo, whi = s * SW_WINS, min((s + 1) * SW_WINS, nwin)
        for qq in range(NCHUNK):
            nt = int(T[qq, wlo:whi].sum())
            batches.append((qq, nt, t0))
            t0 += nt
    assert t0 == ntiles

    meta = dict(nlr=nlr, nl=nl, ntab=ntab, crows=crows, nwin=nwin, n_sw=n_sw,
                ntiles=ntiles, epad=epad, T=T, batches=batches,
                tile_win=tile_win, tile_start=tile_start, tile_stop=tile_stop)

    xT0 = np.zeros((NCORES, P, nl), np.float32)
    xtbl = np.zeros((ntab, D), np.float32)
    for c in range(NCORES):
        xs = x[c * nlr:(c + 1) * nlr]
        xT0[c, :, :nlr] = xs.T
        xtbl[c * nl:c * nl + nlr] = xs

    return meta, idx16, ind, eaT, xT0, xtbl


# ---------------------------------------------------------------------------
# program builder
# ---------------------------------------------------------------------------

def _np_dt(dt):
    return {F32: np.float32, BF16: None, I16: np.int16}[dt]


def _build(meta):
    nl, ntab, crows = meta["nl"], meta["ntab"], meta["crows"]
    nwin, n_sw, ntiles, epad = (meta["nwin"], meta["n_sw"], meta["ntiles"],
                                meta["epad"])
    batches = meta["batches"]
    tile_win = meta["tile_win"]
    tile_start, tile_stop = meta["tile_start"], meta["tile_stop"]

    nc = bacc.Bacc("TRN2", target_bir_lowering=False, debug=False,
                   num_devices=NCORES, num_swdge_queues=4)

    xtbl = nc.dram_tensor("xtbl", [ntab, D], TAB_DT, kind="ExternalInput").ap()
    xT0 = nc.dram_tensor("xT0", [P, nl], F32, kind="ExternalInput").ap()
    idx = nc.dram_tensor("idx", [128, epad // 16], I16, kind="ExternalInput").ap()
    ind = nc.dram_tensor("ind", [P, ntiles * WIN], IND_DT, kind="ExternalInput").ap()
    eaT = nc.dram_tensor("eaT", [ED + 1, epad], F32, kind="ExternalInput").ap()
    wep = nc.dram_tensor("wep", [ED + 1, D], F32, kind="ExternalInput").ap()
    w1s = nc.dram_tensor("w1s", [L, D, D], F32, kind="ExternalInput").ap()
    w2s = nc.dram_tensor("w2s", [L, D, D], F32, kind="ExternalInput").ap()
    b1T = nc.dram_tensor("b1T", [P, L], F32, kind="ExternalInput").ap()
    b2T = nc.dram_tensor("b2T", [P, L], F32, kind="ExternalInput").ap()
    epsT = nc.dram_tensor("epsT", [P, L], F32, kind="ExternalInput").ap()
    gT = nc.dram_tensor("gT", [P, L], F32, kind="ExternalInput").ap()
    bT = nc.dram_tensor("bT", [P, L], F32, kind="ExternalInput").ap()
    mT = nc.dram_tensor("mT", [P, L], F32, kind="ExternalInput").ap()
    vT = nc.dram_tensor("vT", [P, L], F32, kind="ExternalInput").ap()
    out = nc.dram_tensor("out", [nl, D], F32, kind="ExternalOutput").ap()

    ag_in = [nc.dram_tensor(f"agin{l}", [nl, D], TAB_DT).ap()
             for l in range(L - 1)]
    tabs = [nc.dram_tensor(f"tab{l}", [ntab, D], TAB_DT, addr_space="Shared").ap()
            for l in range(L - 1)]

    with tile.TileContext(nc) as tc:
        with (
            tc.tile_pool(name="const", bufs=1) as cpool,
            tc.tile_pool(name="gath", bufs=2) as gpool,
            tc.tile_pool(name="indp", bufs=4) as ipool,
            tc.tile_pool(name="msgp", bufs=4) as mpool,
            tc.tile_pool(name="hp", bufs=3) as hpool,
            tc.tile_pool(name="rows", bufs=4) as rpool,
            tc.tile_pool(name="eap", bufs=2) as eapool,
            tc.tile_pool(name="ps_agg", bufs=2, space="PSUM") as ps_agg,
            tc.tile_pool(name="ps_mlp", bufs=1, space="PSUM") as ps_mlp,
            tc.tile_pool(name="ps_e", bufs=2, space="PSUM") as ps_e,
            tc.tile_pool(name="ps_tr", bufs=1, space="PSUM") as ps_tr,
        ):
            # ---------------- prologue ----------------
            ident = cpool.tile([P, P], F32)
            make_identity(nc, ident[:])
            zero_t = cpool.tile([P, 1], F32)
            nc.vector.memset(zero_t[:], 0.0)

            xT = cpool.tile([P, nl], F32, tag="xT")
            nc.sync.dma_start(out=xT[:], in_=xT0[:])

            idx_t = cpool.tile([128, epad // 16], I16)
            nc.sync.dma_start(out=idx_t[:], in_=idx[:])

            wep_t = cpool.tile([ED + 1, D], F32)
            nc.sync.dma_start(out=wep_t[:], in_=wep[:])

            w1_t = cpool.tile([P, L * D], F32)
            nc.sync.dma_start(out=w1_t[:].rearrange("p (l d) -> p l d", d=D),
                              in_=w1s.rearrange("l a b -> a l b"))
            w2_t = cpool.tile([P, L * D], F32)
            nc.sync.dma_start(out=w2_t[:].rearrange("p (l d) -> p l d", d=D),
                              in_=w2s.rearrange("l a b -> a l b"))

            b1_t = cpool.tile([P, L], F32)
            nc.sync.dma_start(out=b1_t[:], in_=b1T[:])
            b2_t = cpool.tile([P, L], F32)
            nc.sync.dma_start(out=b2_t[:], in_=b2T[:])

            eps_t = cpool.tile([P, L], F32)
            nc.sync.dma_start(out=eps_t[:], in_=epsT[:])
            ep1_t = cpool.tile([P, L], F32)
            nc.vector.tensor_scalar(out=ep1_t[:], in0=eps_t[:], scalar1=1.0,
                                    scalar2=None, op0=mybir.AluOpType.add)

            # BN: scale = g*rsqrt(v+eps); bias' = (beta - mean*scale) + scale*b2
            g_t = cpool.tile([P, L], F32)
            nc.sync.dma_start(out=g_t[:], in_=gT[:])
            be_t = cpool.tile([P, L], F32)
            nc.sync.dma_start(out=be_t[:], in_=bT[:])
            m_t = cpool.tile([P, L], F32)
            nc.sync.dma_start(out=m_t[:], in_=mT[:])
            v_t = cpool.tile([P, L], F32)
            nc.sync.dma_start(out=v_t[:], in_=vT[:])
            epsc_t = cpool.tile([P, 1], F32)
            nc.vector.memset(epsc_t[:], BN_EPS)
            sd_t = cpool.tile([P, L], F32)
            nc.scalar.activation(sd_t[:], v_t[:],
                                 mybir.ActivationFunctionType.Sqrt,
                                 bias=epsc_t[:])
            rs_t = cpool.tile([P, L], F32)
            nc.vector.reciprocal(rs_t[:], sd_t[:])
            bns_t = cpool.tile([P, L], F32)
            nc.vector.tensor_tensor(out=bns_t[:], in0=g_t[:], in1=rs_t[:],
                                    op=mybir.AluOpType.mult)
            tmp_t = cpool.tile([P, L], F32)
            nc.vector.tensor_tensor(out=tmp_t[:], in0=m_t[:], in1=bns_t[:],
                                    op=mybir.AluOpType.mult)
            bnb_t = cpool.tile([P, L], F32)
            nc.vector.tensor_tensor(out=bnb_t[:], in0=be_t[:], in1=tmp_t[:],
                                    op=mybir.AluOpType.subtract)
            tmp2_t = cpool.tile([P, L], F32)
            nc.vector.tensor_tensor(out=tmp2_t[:], in0=b2_t[:], in1=bns_t[:],
                                    op=mybir.AluOpType.mult)
            bb2_t = cpool.tile([P, L], F32)
            nc.vector.tensor_tensor(out=bb2_t[:], in0=bnb_t[:], in1=tmp2_t[:],
                                    op=mybir.AluOpType.add)

            # ---------------- layers ----------------
            for l in range(L):
                table = xtbl if l == 0 else tabs[l - 1]
                for s in range(n_sw):
                    wlo = s * SW_WINS
                    whi = min(wlo + SW_WINS, nwin)
                    nw = whi - wlo
                    cn = nw * WIN

                    # load + msg for the 4 chunk batches of this superwindow
                    mbs = []
                    ibs = []
                    for qq in range(NCHUNK):
                        q2, nt, tlo = batches[s * NCHUNK + qq]
                        assert q2 == qq
                        gb = gpool.tile([P, GTILES, D], TAB_DT, tag="gb")
                        nc.gpsimd.dma_gather(
                            out_ap=gb[:, :nt, :],
                            in_ap=table[qq * crows:(qq + 1) * crows, :],
                            idxs_ap=idx_t[:, tlo * 8:(tlo + nt) * 8],
                            num_idxs=nt * P, num_idxs_reg=nt * P, elem_size=D,
                            queue_num=qq)
                        ib = ipool.tile([P, GTILES * WIN], IND_DT, tag="ib",
                                        name=f"ib_{l}_{s}_{qq}")
                        nc.sync.dma_start(out=ib[:, :nt * WIN],
                                          in_=ind[:, tlo * WIN:(tlo + nt) * WIN])
                        ea_t = eapool.tile([ED + 1, GTILES * P], F32, tag="ea",
                                           name=f"ea_{l}_{s}_{qq}")
                        nc.sync.dma_start(out=ea_t[:, :nt * P],
                                          in_=eaT[:, tlo * P:(tlo + nt) * P])
                        mb = mpool.tile([P, GTILES * D], MSG_DT, tag="mb",
                                        name=f"mb_{l}_{s}_{qq}")
                        for g0 in range(0, nt, 4):
                            gn = min(4, nt - g0)
                            pe4 = ps_e.tile([P, 4 * P], F32, space="PSUM",
                                            tag="pse",
                                            name=f"pse_{l}_{s}_{qq}_{g0}")
                            for j in range(gn):
                                nc.tensor.matmul(
                                    pe4[:, j * P:(j + 1) * P],
                                    lhsT=ea_t[:, (g0 + j) * P:(g0 + j + 1) * P],
                                    rhs=wep_t[:], start=True, stop=True)
                            nc.vector.tensor_tensor(
                                out=mb[:, g0 * D:(g0 + gn) * D],
                                in0=gb[:, g0:g0 + gn, :].rearrange(
                                    "p t d -> p (t d)"),
                                in1=pe4[:, :gn * P],
                                op=mybir.AluOpType.add)
                        nc.scalar.activation(mb[:, :nt * D], mb[:, :nt * D],
                                             mybir.ActivationFunctionType.Relu,
                                             bias=zero_t[:])
                        mbs.append((mb, tlo, nt))
                        ibs.append(ib)

                    # window-major matmuls: one accumulation group per window
                    ap_t = ps_agg.tile([P, SW_WINS * WIN], F32, space="PSUM",
                                       tag="agg", name=f"agg_{l}_{s}")
                    for wl in range(nw):
                        for qq in range(NCHUNK):
                            mb, tlo, nt = mbs[qq]
                            ib = ibs[qq]
                            for j in range(nt):
                                t = tlo + j
                                if tile_win[t] != wlo + wl:
                                    continue
                                nc.tensor.matmul(
                                    ap_t[:, wl * WIN:(wl + 1) * WIN],
                                    lhsT=mb[:, j * D:(j + 1) * D],
                                    rhs=ib[:, j * WIN:(j + 1) * WIN],
                                    start=bool(tile_start[t]),
                                    stop=bool(tile_stop[t]))

                    # h = (1+eps)x + agg ; MLP in chunks of 4 windows
                    for c4 in range(0, nw, 4):
                        cw = min(4, nw - c4)
                        ccn = cw * WIN
                        co = (wlo + c4) * WIN
                        hT = hpool.tile([P, 4 * WIN], F32, tag="hT")
                        nc.vector.tensor_scalar(
                            out=hT[:, :ccn], in0=xT[:, co:co + ccn],
                            scalar1=ep1_t[:, l:l + 1], scalar2=None,
                            op0=mybir.AluOpType.mult)
                        nc.vector.tensor_tensor(
                            out=hT[:, :ccn], in0=hT[:, :ccn],
                            in1=ap_t[:, c4 * WIN:c4 * WIN + ccn],
                            op=mybir.AluOpType.add)
                        p1 = ps_mlp.tile([P, 4 * WIN], F32, space="PSUM",
                                         tag="p1")
                        nc.tensor.matmul(p1[:, :ccn],
                                         lhsT=w1_t[:, l * D:(l + 1) * D],
                                         rhs=hT[:, :ccn], start=True, stop=True)
                        h1 = hpool.tile([P, 4 * WIN], F32, tag="h1")
                        nc.scalar.activation(h1[:, :ccn], p1[:, :ccn],
                                             mybir.ActivationFunctionType.Relu,
                                             bias=b1_t[:, l:l + 1])
                        p2 = ps_mlp.tile([P, 4 * WIN], F32, space="PSUM",
                                         tag="p2")
                        nc.tensor.matmul(p2[:, :ccn],
                                         lhsT=w2_t[:, l * D:(l + 1) * D],
                                         rhs=h1[:, :ccn], start=True, stop=True)
                        yT = hpool.tile([P, 4 * WIN], F32, tag="yT")
                        nc.scalar.activation(yT[:, :ccn], p2[:, :ccn],
                                             mybir.ActivationFunctionType.Relu,
                                             scale=bns_t[:, l:l + 1],
                                             bias=bb2_t[:, l:l + 1])
                        nc.vector.tensor_tensor(
                            out=xT[:, co:co + ccn],
                            in0=xT[:, co:co + ccn],
                            in1=yT[:, :ccn], op=mybir.AluOpType.add)

                # transpose xT -> rows; AG or final output
                dst = out if l == L - 1 else ag_in[l]
                for b in range(nl // P):
                    tp = ps_tr.tile([P, P], F32, space="PSUM", tag="tp")
                    nc.tensor.transpose(out=tp[:], in_=xT[:, b * P:(b + 1) * P],
                                        identity=ident[:])
                    rt = rpool.tile([P, P], F32 if l == L - 1 else TAB_DT,
                                    tag="rt")
                    nc.vector.tensor_copy(rt[:], tp[:])
                    nc.sync.dma_start(out=dst[b * P:(b + 1) * P, :], in_=rt[:])
                if l < L - 1:
                    nc.gpsimd.collective_compute(
                        "AllGather", mybir.AluOpType.bypass,
                        replica_groups=[list(range(NCORES))],
                        ins=[ag_in[l][:].opt()], outs=[tabs[l][:].opt()])

    nc.compile()
    return nc


# ---------------------------------------------------------------------------
# entry point
# ---------------------------------------------------------------------------

_CACHE = {}


def _to_bf16(a):
    import ml_dtypes
    return a.astype(ml_dtypes.bfloat16)


def kernel(x, edge_index, edge_attr, edge_weight, We, be, W1, b1, W2, b2,
           eps, gamma, beta, run_mean, run_var):
    x = np.asarray(x, np.float32)
    edge_index = np.asarray(edge_index)
    edge_attr = np.asarray(edge_attr, np.float32)
    edge_weight = np.asarray(edge_weight, np.float32)
    n_nodes = x.shape[0]

    meta, idx16, ind_a, eaT_a, xT0_a, xtbl_a = _prep(
        x, edge_index, edge_attr, edge_weight, n_nodes)

    key = (n_nodes, meta["epad"], tuple(int(v) for v in meta["T"].ravel()))
    if key not in _CACHE:
        _CACHE[key] = _build(meta)
    nc = _CACHE[key]

    wep = np.concatenate([np.asarray(We, np.float32),
                          np.asarray(be, np.float32)[None, :]], axis=0)
    shared = {
        "wep": wep,
        "w1s": np.asarray(W1, np.float32),
        "w2s": np.asarray(W2, np.float32),
        "b1T": np.ascontiguousarray(np.asarray(b1, np.float32).T),
        "b2T": np.ascontiguousarray(np.asarray(b2, np.float32).T),
        "epsT": np.tile(np.asarray(eps, np.float32)[None, :], (P, 1)),
        "gT": np.ascontiguousarray(np.asarray(gamma, np.float32).T),
        "bT": np.ascontiguousarray(np.asarray(beta, np.float32).T),
        "mT": np.ascontiguousarray(np.asarray(run_mean, np.float32).T),
        "vT": np.ascontiguousarray(np.asarray(run_var, np.float32).T),
    }
    in_maps = []
    for c in range(NCORES):
        m = dict(shared)
        m["xtbl"] = _to_bf16(xtbl_a) if TAB_DT == BF16 else xtbl_a
        m["xT0"] = xT0_a[c]
        m["idx"] = idx16[c]
        m["ind"] = (_to_bf16(ind_a[c].reshape(P, -1)) if IND_DT == BF16
                    else ind_a[c].reshape(P, -1))
        m["eaT"] = eaT_a[c]
        in_maps.append(m)

    trace = False
    import os
    if os.environ.get("GINE_TRACE") == "1":
        try:
            import sys
            import types
            from trn_agent_boot.trn_boot import _ntff_profile_via_ctypes
            hook = _ntff_profile_via_ctypes("/opt/axon/libaxon_pjrt.so")
            mod = types.ModuleType("antenv.axon_hooks")
            mod.get_axon_ntff_profile_hook = lambda: hook
            mod.set_axon_ntff_profile_hook = lambda h: None
            sys.modules["antenv.axon_hooks"] = mod
            trace = True
        except Exception:
            trace = False

    r = run_bass_kernel_spmd(nc, in_maps, list(range(NCORES)), trace=trace)
    global LAST_RESULT
    LAST_RESULT = r
    nlr = meta["nlr"]
    return np.concatenate([r.results[c]["out"][:nlr] for c in range(NCORES)],
                          axis=0)



# revision 4
# speedup vs baseline: 1.1573x; 1.1573x over previous
"""Trainium2 Bass kernel for BaseGINE (4-layer GINE message-passing GNN).

Self-contained: takes full inputs, shards across 8 NeuronCores internally,
returns the full output.

Sharding: nodes are partitioned contiguously across the 8 cores (12500 each,
padded to 12544 = 98*128). Edges are assigned to the core owning their dst
node and sorted into (superwindow, src-chunk) cells:
  - superwindow: 4 consecutive 128-node dst windows (one PSUM bank of agg)
  - src-chunk: quarter of the padded node table holding the src row
    (dma_gather indices are int16, so the 100352-row table is read in 4
    chunks of 25088 rows)
Within a cell edges are sorted by dst; 128-edge tiles therefore span 1-2 dst
windows and each tile contributes one [128,128] weighted-indicator matmul per
touched window (host computes a core-uniform block schedule; a core whose
tile misses a scheduled window gets an all-zero indicator block). Pad slots
at the end of each cell use negative gather indices (skipped by the DMA
gather ucode) and zero indicator columns.

Per layer, per superwindow: dma_gather x[src] for the 4 chunk cells ->
msg = relu(x_src + e_emb) (bf16) -> per-block indicator matmuls accumulate
aggT[d, window] in one PSUM bank -> h = (1+eps)x + agg -> MLP/BN/relu/
residual on the x^T-resident fp32 slice. Layer end: PE-transpose x^T to
rows -> AllGather rebuilds the gather table for the next layer.
"""

import numpy as np

import concourse.bass as bass
import concourse.bacc as bacc
import concourse.mybir as mybir
import concourse.tile as tile
from concourse.bass_utils import run_bass_kernel_spmd
from concourse.masks import make_identity

F32 = mybir.dt.float32
BF16 = mybir.dt.bfloat16
I16 = mybir.dt.int16

NCORES = 8
D = 128
ED = 16
L = 4
P = 128
WIN = 128            # dst-window width (nodes)
SW_WINS = 4          # windows per superwindow (one PSUM bank)
NCHUNK = 4           # gather-table chunks (int16 index limit)
GTILES = 24          # max tiles per dma_gather instruction
BN_EPS = 1e-5
EMB_DT = BF16        # e_emb storage dtype
IND_DT = BF16        # indicator dtype (matches msg dtype for matmul)
MSG_DT = BF16        # msg dtype (agg matmul operands)
TAB_DT = BF16        # gather table dtype
EA_DT = BF16         # edge_attr (e_emb matmul operand) dtype


# ---------------------------------------------------------------------------
# host-side prep
# ---------------------------------------------------------------------------

def _prep(x, edge_index, edge_attr, edge_weight, n_nodes):
    """Sort/pad edges into the (superwindow, chunk) cell structure."""
    nlr = n_nodes // NCORES              # real nodes per core
    nl = -(-nlr // P) * P                # padded nodes per core
    ntab = nl * NCORES                   # padded table rows
    crows = ntab // NCHUNK               # table rows per chunk
    assert crows <= 32768
    nwin = nl // WIN                     # windows per core
    n_sw = -(-nwin // SW_WINS)           # superwindows per core

    src, dst = edge_index[0].astype(np.int64), edge_index[1].astype(np.int64)
    core = dst // nlr
    ldst = dst - core * nlr
    swi = (ldst // WIN) // SW_WINS
    spos = (src // nlr) * nl + (src % nlr)   # src row in padded table
    q = spos // crows

    order = np.lexsort((ldst, q, swi, core))
    sc, ssw, sq, sld = core[order], swi[order], q[order], ldst[order]

    counts = np.zeros((NCORES, n_sw, NCHUNK), np.int64)
    for c in range(NCORES):
        m = sc == c
        np.add.at(counts[c], (ssw[m], sq[m]), 1)
    T = -(-counts.max(axis=0) // P)          # [n_sw, NCHUNK]

    # cell tile offsets in stream order (sw -> chunk)
    tlo_map = np.zeros((n_sw, NCHUNK), np.int64)
    t0 = 0
    for s in range(n_sw):
        for qq in range(NCHUNK):
            tlo_map[s, qq] = t0
            t0 += int(T[s, qq])
    ntiles = t0
    epad = ntiles * P

    idx16 = np.zeros((NCORES, 128, epad // 16), np.int16)
    eaT = np.zeros((NCORES, ED + 1, epad), np.float32)
    dstw = np.full((NCORES, epad), -1, np.int64)    # local window in sw
    dstc = np.zeros((NCORES, epad), np.int64)       # col within window
    eww = np.zeros((NCORES, epad), np.float32)
    touch = np.zeros((ntiles, SW_WINS), bool)       # union over cores

    key = (sc * n_sw + ssw) * NCHUNK + sq
    bounds = np.flatnonzero(np.r_[True, key[1:] != key[:-1]])
    bounds = np.r_[bounds, len(order)]
    for b0, b1 in zip(bounds[:-1], bounds[1:]):
        c, s, qq = int(sc[b0]), int(ssw[b0]), int(sq[b0])
        e = order[b0:b1]
        k = len(e)
        t0 = int(tlo_map[s, qq])
        s0 = t0 * P
        i = np.arange(k)
        gs = s0 + i
        idx16[c, gs % 16, gs // 16] = (spos[e] - qq * crows).astype(np.int16)
        eaT[c, :ED, s0:s0 + k] = edge_attr[e].T
        eaT[c, ED, s0:s0 + k] = 1.0
        lw = sld[b0:b1] // WIN - s * SW_WINS
        dstw[c, s0:s0 + k] = lw
        dstc[c, s0:s0 + k] = sld[b0:b1] % WIN
        eww[c, s0:s0 + k] = edge_weight[e]
        touch[t0 + i // P, lw] = True
    for g in range(1, 8):
        idx16[:, g * 16:(g + 1) * 16, :] = idx16[:, :16, :]

    # core-uniform block schedule
    batches = []     # stream order: (sw, chunk)
    blo = 0
    for s in range(n_sw):
        for qq in range(NCHUNK):
            nt = int(T[s, qq])
            t0 = int(tlo_map[s, qq])
            bl = []
            for j in range(nt):
                for w in np.flatnonzero(touch[t0 + j]):
                    bl.append((j, int(w)))
            batches.append(dict(q=qq, s=s, nt=nt, tlo=t0, blo=blo, blocks=bl))
            blo += len(bl)
            assert nt <= GTILES and len(bl) <= GTILES
    nblocks = blo

    # start/stop flags per sw per window
    blk_start = np.zeros(max(nblocks, 1), bool)
    blk_stop = np.zeros(max(nblocks, 1), bool)
    for s in range(n_sw):
        first, last = {}, {}
        for b in batches[s * NCHUNK:(s + 1) * NCHUNK]:
            for kblk, (j, w) in enumerate(b["blocks"]):
                g = b["blo"] + kblk
                if w not in first:
                    first[w] = g
                last[w] = g
        wlim = min(SW_WINS, nwin - s * SW_WINS)
        assert set(first) == set(range(wlim)), (s, sorted(first))
        for w in first:
            blk_start[first[w]] = True
            blk_stop[last[w]] = True

    ind = np.zeros((NCORES, P, nblocks, WIN), np.float32)
    for b in batches:
        for kblk, (j, w) in enumerate(b["blocks"]):
            g = b["blo"] + kblk
            sl = slice((b["tlo"] + j) * P, (b["tlo"] + j + 1) * P)
            for c in range(NCORES):
                pp = np.flatnonzero(dstw[c, sl] == w)
                ind[c, pp, g, dstc[c, sl][pp]] = eww[c, sl][pp]

    meta = dict(nlr=nlr, nl=nl, ntab=ntab, crows=crows, nwin=nwin, n_sw=n_sw,
                ntiles=ntiles, epad=epad, nblocks=nblocks, batches=batches,
                blk_start=blk_start, blk_stop=blk_stop)

    xT0 = np.zeros((NCORES, P, nl), np.float32)
    xtbl = np.zeros((ntab, D), np.float32)
    for c in range(NCORES):
        xs = x[c * nlr:(c + 1) * nlr]
        xT0[c, :, :nlr] = xs.T
        xtbl[c * nl:c * nl + nlr] = xs

    return meta, idx16, ind, eaT, xT0, xtbl


# ---------------------------------------------------------------------------
# program builder
# ---------------------------------------------------------------------------

def _build(meta):
    nl, ntab, crows = meta["nl"], meta["ntab"], meta["crows"]
    nwin, n_sw, ntiles, epad = (meta["nwin"], meta["n_sw"], meta["ntiles"],
                                meta["epad"])
    nblocks = meta["nblocks"]
    batches = meta["batches"]
    blk_start, blk_stop = meta["blk_start"], meta["blk_stop"]

    nc = bacc.Bacc("TRN2", target_bir_lowering=False, debug=False,
                   num_devices=NCORES, num_swdge_queues=4)

    xtbl = nc.dram_tensor("xtbl", [ntab, D], TAB_DT, kind="ExternalInput").ap()
    xT0 = nc.dram_tensor("xT0", [P, nl], F32, kind="ExternalInput").ap()
    idx = nc.dram_tensor("idx", [128, epad // 16], I16, kind="ExternalInput").ap()
    ind = nc.dram_tensor("ind", [P, nblocks * WIN], IND_DT,
                         kind="ExternalInput").ap()
    eaT = nc.dram_tensor("eaT", [ED + 1, epad], EA_DT, kind="ExternalInput").ap()
    wep = nc.dram_tensor("wep", [ED + 1, D], EA_DT, kind="ExternalInput").ap()
    w1s = nc.dram_tensor("w1s", [L, D, D], F32, kind="ExternalInput").ap()
    w2s = nc.dram_tensor("w2s", [L, D, D], F32, kind="ExternalInput").ap()
    b1T = nc.dram_tensor("b1T", [P, L], F32, kind="ExternalInput").ap()
    b2T = nc.dram_tensor("b2T", [P, L], F32, kind="ExternalInput").ap()
    epsT = nc.dram_tensor("epsT", [P, L], F32, kind="ExternalInput").ap()
    gT = nc.dram_tensor("gT", [P, L], F32, kind="ExternalInput").ap()
    bT = nc.dram_tensor("bT", [P, L], F32, kind="ExternalInput").ap()
    mT = nc.dram_tensor("mT", [P, L], F32, kind="ExternalInput").ap()
    vT = nc.dram_tensor("vT", [P, L], F32, kind="ExternalInput").ap()
    out = nc.dram_tensor("out", [nl, D], F32, kind="ExternalOutput").ap()

    ag_in = [nc.dram_tensor(f"agin{l}", [nl, D], TAB_DT).ap()
             for l in range(L - 1)]
    tabs = [nc.dram_tensor(f"tab{l}", [ntab, D], TAB_DT, addr_space="Shared").ap()
            for l in range(L - 1)]

    with tile.TileContext(nc) as tc:
        with (
            tc.tile_pool(name="const", bufs=1) as cpool,
            tc.tile_pool(name="gath", bufs=2) as gpool,
            tc.tile_pool(name="indp", bufs=4) as ipool,
            tc.tile_pool(name="msgp", bufs=4) as mpool,
            tc.tile_pool(name="hp", bufs=3) as hpool,
            tc.tile_pool(name="rows", bufs=4) as rpool,
            tc.tile_pool(name="eap", bufs=2) as eapool,
            tc.tile_pool(name="ps_agg", bufs=2, space="PSUM") as ps_agg,
            tc.tile_pool(name="ps_mlp", bufs=1, space="PSUM") as ps_mlp,
            tc.tile_pool(name="ps_e", bufs=2, space="PSUM") as ps_e,
            tc.tile_pool(name="ps_tr", bufs=1, space="PSUM") as ps_tr,
        ):
            # ---------------- prologue ----------------
            ident = cpool.tile([P, P], F32)
            make_identity(nc, ident[:])
            zero_t = cpool.tile([P, 1], F32)
            nc.vector.memset(zero_t[:], 0.0)

            # pad gather slots are skipped by the DMA (negative idx) and
            # read stale SBUF: zero both gather buffers once so stale data
            # is always finite (indicator zeros take care of the rest)
            for _ in range(2):
                gz = gpool.tile([P, GTILES, D], TAB_DT, tag="gb")
                nc.vector.memset(gz[:].rearrange("p t d -> p (t d)"), 0.0)

            xT = cpool.tile([P, nl], F32, tag="xT")
            nc.sync.dma_start(out=xT[:], in_=xT0[:])

            idx_t = cpool.tile([128, epad // 16], I16)
            nc.sync.dma_start(out=idx_t[:], in_=idx[:])

            wep_t = cpool.tile([ED + 1, D], EA_DT)
            nc.sync.dma_start(out=wep_t[:], in_=wep[:])

            w1_t = cpool.tile([P, L * D], F32)
            nc.sync.dma_start(out=w1_t[:].rearrange("p (l d) -> p l d", d=D),
                              in_=w1s.rearrange("l a b -> a l b"))
            w2_t = cpool.tile([P, L * D], F32)
            nc.sync.dma_start(out=w2_t[:].rearrange("p (l d) -> p l d", d=D),
                              in_=w2s.rearrange("l a b -> a l b"))

            b1_t = cpool.tile([P, L], F32)
            nc.sync.dma_start(out=b1_t[:], in_=b1T[:])
            b2_t = cpool.tile([P, L], F32)
            nc.sync.dma_start(out=b2_t[:], in_=b2T[:])

            eps_t = cpool.tile([P, L], F32)
            nc.sync.dma_start(out=eps_t[:], in_=epsT[:])
            ep1_t = cpool.tile([P, L], F32)
            nc.vector.tensor_scalar(out=ep1_t[:], in0=eps_t[:], scalar1=1.0,
                                    scalar2=None, op0=mybir.AluOpType.add)

            # BN: scale = g*rsqrt(v+eps); bias' = (beta - mean*scale) + scale*b2
            g_t = cpool.tile([P, L], F32)
            nc.sync.dma_start(out=g_t[:], in_=gT[:])
            be_t = cpool.tile([P, L], F32)
            nc.sync.dma_start(out=be_t[:], in_=bT[:])
            m_t = cpool.tile([P, L], F32)
            nc.sync.dma_start(out=m_t[:], in_=mT[:])
            v_t = cpool.tile([P, L], F32)
            nc.sync.dma_start(out=v_t[:], in_=vT[:])
            epsc_t = cpool.tile([P, 1], F32)
            nc.vector.memset(epsc_t[:], BN_EPS)
            sd_t = cpool.tile([P, L], F32)
            nc.scalar.activation(sd_t[:], v_t[:],
                                 mybir.ActivationFunctionType.Sqrt,
                                 bias=epsc_t[:])
            rs_t = cpool.tile([P, L], F32)
            nc.vector.reciprocal(rs_t[:], sd_t[:])
            bns_t = cpool.tile([P, L], F32)
            nc.vector.tensor_tensor(out=bns_t[:], in0=g_t[:], in1=rs_t[:],
                                    op=mybir.AluOpType.mult)
            tmp_t = cpool.tile([P, L], F32)
            nc.vector.tensor_tensor(out=tmp_t[:], in0=m_t[:], in1=bns_t[:],
                                    op=mybir.AluOpType.mult)
            bnb_t = cpool.tile([P, L], F32)
            nc.vector.tensor_tensor(out=bnb_t[:], in0=be_t[:], in1=tmp_t[:],
                                    op=mybir.AluOpType.subtract)
            tmp2_t = cpool.tile([P, L], F32)
            nc.vector.tensor_tensor(out=tmp2_t[:], in0=b2_t[:], in1=bns_t[:],
                                    op=mybir.AluOpType.mult)
            bb2_t = cpool.tile([P, L], F32)
            nc.vector.tensor_tensor(out=bb2_t[:], in0=bnb_t[:], in1=tmp2_t[:],
                                    op=mybir.AluOpType.add)

            # ---------------- layers ----------------
            for l in range(L):
                table = xtbl if l == 0 else tabs[l - 1]
                for s in range(n_sw):
                    wlo = s * SW_WINS
                    whi = min(wlo + SW_WINS, nwin)
                    nw = whi - wlo

                    # load + msg for the 4 chunk cells of this superwindow
                    mbs = []
                    ibs = []
                    for qq in range(NCHUNK):
                        b = batches[s * NCHUNK + qq]
                        nt, tlo, nblk, blo = (b["nt"], b["tlo"],
                                              len(b["blocks"]), b["blo"])
                        if nt == 0:
                            mbs.append(None)
                            ibs.append(None)
                            continue
                        gb = gpool.tile([P, GTILES, D], TAB_DT, tag="gb")
                        nc.gpsimd.dma_gather(
                            out_ap=gb[:, :nt, :],
                            in_ap=table[qq * crows:(qq + 1) * crows, :],
                            idxs_ap=idx_t[:, tlo * 8:(tlo + nt) * 8],
                            num_idxs=nt * P, num_idxs_reg=nt * P, elem_size=D,
                            queue_num=qq)
                        ib = ipool.tile([P, GTILES * WIN], IND_DT, tag="ib",
                                        name=f"ib_{l}_{s}_{qq}")
                        nc.sync.dma_start(out=ib[:, :nblk * WIN],
                                          in_=ind[:, blo * WIN:(blo + nblk) * WIN])
                        ea_t = eapool.tile([ED + 1, GTILES * P], EA_DT, tag="ea",
                                           name=f"ea_{l}_{s}_{qq}")
                        nc.sync.dma_start(out=ea_t[:, :nt * P],
                                          in_=eaT[:, tlo * P:(tlo + nt) * P])
                        mb = mpool.tile([P, GTILES * D], MSG_DT, tag="mb",
                                        name=f"mb_{l}_{s}_{qq}")
                        for g0 in range(0, nt, 4):
                            gn = min(4, nt - g0)
                            pe4 = ps_e.tile([P, 4 * P], F32, space="PSUM",
                                            tag="pse",
                                            name=f"pse_{l}_{s}_{qq}_{g0}")
                            for j in range(gn):
                                nc.tensor.matmul(
                                    pe4[:, j * P:(j + 1) * P],
                                    lhsT=ea_t[:, (g0 + j) * P:(g0 + j + 1) * P],
                                    rhs=wep_t[:], start=True, stop=True)
                            nc.vector.tensor_tensor(
                                out=mb[:, g0 * D:(g0 + gn) * D],
                                in0=gb[:, g0:g0 + gn, :].rearrange(
                                    "p t d -> p (t d)"),
                                in1=pe4[:, :gn * P],
                                op=mybir.AluOpType.add)
                        nc.scalar.activation(mb[:, :nt * D], mb[:, :nt * D],
                                             mybir.ActivationFunctionType.Relu,
                                             bias=zero_t[:])
                        mbs.append(mb)
                        ibs.append(ib)

                    # indicator matmuls: window-major so each window's PSUM
                    # accumulation group is contiguous in PE issue order
                    ap_t = ps_agg.tile([P, SW_WINS * WIN], F32, space="PSUM",
                                       tag="agg", name=f"agg_{l}_{s}")
                    for w in range(nw):
                        wb = []
                        for qq in range(NCHUNK):
                            b = batches[s * NCHUNK + qq]
                            for kblk, (j, bw) in enumerate(b["blocks"]):
                                if bw == w:
                                    wb.append((qq, kblk, j))
                        for i, (qq, kblk, j) in enumerate(wb):
                            nc.tensor.matmul(
                                ap_t[:, w * WIN:(w + 1) * WIN],
                                lhsT=mbs[qq][:, j * D:(j + 1) * D],
                                rhs=ibs[qq][:, kblk * WIN:(kblk + 1) * WIN],
                                start=(i == 0),
                                stop=(i == len(wb) - 1))

                    # h = (1+eps)x + agg ; MLP in chunks of 4 windows
                    for c4 in range(0, nw, 4):
                        cw = min(4, nw - c4)
                        ccn = cw * WIN
                        co = (wlo + c4) * WIN
                        hT = hpool.tile([P, 4 * WIN], F32, tag="hT")
                        nc.vector.tensor_scalar(
                            out=hT[:, :ccn], in0=xT[:, co:co + ccn],
                            scalar1=ep1_t[:, l:l + 1], scalar2=None,
                            op0=mybir.AluOpType.mult)
                        nc.vector.tensor_tensor(
                            out=hT[:, :ccn], in0=hT[:, :ccn],
                            in1=ap_t[:, c4 * WIN:c4 * WIN + ccn],
                            op=mybir.AluOpType.add)
                        p1 = ps_mlp.tile([P, 4 * WIN], F32, space="PSUM",
                                         tag="p1")
                        nc.tensor.matmul(p1[:, :ccn],
                                         lhsT=w1_t[:, l * D:(l + 1) * D],
                                         rhs=hT[:, :ccn], start=True, stop=True)
                        h1 = hpool.tile([P, 4 * WIN], F32, tag="h1")
                        nc.scalar.activation(h1[:, :ccn], p1[:, :ccn],
                                             mybir.ActivationFunctionType.Relu,
                                             bias=b1_t[:, l:l + 1])
                        p2 = ps_mlp.tile([P, 4 * WIN], F32, space="PSUM",
                                         tag="p2")
                        nc.tensor.matmul(p2[:, :ccn],
                                         lhsT=w2_t[:, l * D:(l + 1) * D],
                                         rhs=h1[:, :ccn], start=True, stop=True)
                        yT = hpool.tile([P, 4 * WIN], F32, tag="yT")
                        nc.scalar.activation(yT[:, :ccn], p2[:, :ccn],
                                             mybir.ActivationFunctionType.Relu,
                                             scale=bns_t[:, l:l + 1],
                                             bias=bb2_t[:, l:l + 1])
                        nc.vector.tensor_tensor(
                            out=xT[:, co:co + ccn],
                            in0=xT[:, co:co + ccn],
                            in1=yT[:, :ccn], op=mybir.AluOpType.add)

                # transpose xT -> rows; AG or final output
                dst = out if l == L - 1 else ag_in[l]
                for b in range(nl // P):
                    tp = ps_tr.tile([P, P], F32, space="PSUM", tag="tp")
                    nc.tensor.transpose(out=tp[:], in_=xT[:, b * P:(b + 1) * P],
                                        identity=ident[:])
                    rt = rpool.tile([P, P], F32 if l == L - 1 else TAB_DT,
                                    tag="rt")
                    nc.vector.tensor_copy(rt[:], tp[:])
                    nc.sync.dma_start(out=dst[b * P:(b + 1) * P, :], in_=rt[:])
                if l < L - 1:
                    nc.gpsimd.collective_compute(
                        "AllGather", mybir.AluOpType.bypass,
                        replica_groups=[list(range(NCORES))],
                        ins=[ag_in[l][:].opt()], outs=[tabs[l][:].opt()])

    nc.compile()
    return nc


# ---------------------------------------------------------------------------
# entry point
# ---------------------------------------------------------------------------

_CACHE = {}


def _to_bf16(a):
    import ml_dtypes
    return a.astype(ml_dtypes.bfloat16)


def kernel(x, edge_index, edge_attr, edge_weight, We, be, W1, b1, W2, b2,
           eps, gamma, beta, run_mean, run_var):
    x = np.asarray(x, np.float32)
    edge_index = np.asarray(edge_index)
    edge_attr = np.asarray(edge_attr, np.float32)
    edge_weight = np.asarray(edge_weight, np.float32)
    n_nodes = x.shape[0]

    meta, idx16, ind_a, eaT_a, xT0_a, xtbl_a = _prep(
        x, edge_index, edge_attr, edge_weight, n_nodes)

    key = (n_nodes, meta["epad"], meta["nblocks"],
           tuple(b["nt"] for b in meta["batches"]),
           tuple(len(b["blocks"]) for b in meta["batches"]))
    if key not in _CACHE:
        _CACHE[key] = _build(meta)
    nc = _CACHE[key]

    wep = np.concatenate([np.asarray(We, np.float32),
                          np.asarray(be, np.float32)[None, :]], axis=0)
    shared = {
        "wep": _to_bf16(wep) if EA_DT == BF16 else wep,
        "w1s": np.asarray(W1, np.float32),
        "w2s": np.asarray(W2, np.float32),
        "b1T": np.ascontiguousarray(np.asarray(b1, np.float32).T),
        "b2T": np.ascontiguousarray(np.asarray(b2, np.float32).T),
        "epsT": np.tile(np.asarray(eps, np.float32)[None, :], (P, 1)),
        "gT": np.ascontiguousarray(np.asarray(gamma, np.float32).T),
        "bT": np.ascontiguousarray(np.asarray(beta, np.float32).T),
        "mT": np.ascontiguousarray(np.asarray(run_mean, np.float32).T),
        "vT": np.ascontiguousarray(np.asarray(run_var, np.float32).T),
    }
    in_maps = []
    for c in range(NCORES):
        m = dict(shared)
        m["xtbl"] = _to_bf16(xtbl_a) if TAB_DT == BF16 else xtbl_a
        m["xT0"] = xT0_a[c]
        m["idx"] = idx16[c]
        m["ind"] = (_to_bf16(ind_a[c].reshape(P, -1)) if IND_DT == BF16
                    else ind_a[c].reshape(P, -1))
        m["eaT"] = _to_bf16(eaT_a[c]) if EA_DT == BF16 else eaT_a[c]
        in_maps.append(m)

    trace = False
    import os
    if os.environ.get("GINE_TRACE") == "1":
        try:
            import sys
            import types
            from trn_agent_boot.trn_boot import _ntff_profile_via_ctypes
            hook = _ntff_profile_via_ctypes("/opt/axon/libaxon_pjrt.so")
            mod = types.ModuleType("antenv.axon_hooks")
            mod.get_axon_ntff_profile_hook = lambda: hook
            mod.set_axon_ntff_profile_hook = lambda h: None
            sys.modules["antenv.axon_hooks"] = mod
            trace = True
        except Exception:
            trace = False

    r = run_bass_kernel_spmd(nc, in_maps, list(range(NCORES)), trace=trace)
    global LAST_RESULT
    LAST_RESULT = r
    nlr = meta["nlr"]
    return np.concatenate([r.results[c]["out"][:nlr] for c in range(NCORES)],
                          axis=0)
